# revision 15
# baseline (speedup 1.0000x reference)
"""BiLSTM parser kernel for 8 TRN2 NeuronCores.

Strategy (per sharding hint): the sequential 2-layer BiLSTM is replicated on
every core (fwd/bwd interleaved on one core's engines); the n x n pair grid
is sharded by head rows: core i computes score[64*i : 64*i+64, :] using its
partition_id to slice the u-matrix columns. Host gathers the 8 row blocks.

Layouts are feature-on-partition ("transposed") throughout:
  - x.T / lstm_out.T tiles: [128 feat, 512 t]
  - recurrent matmul: weights stationary (orientation B), gates land in one
    PSUM bank as 8 columns [128, 8]; col order (i0,i1,f0,f1,o0,o1,g0,g1) so
    sigmoid covers cols 0:6 in one ACT and tanh cols 6:8.
  - gates_in precomputed batched, stored t-major-interleaved [128, (t,8)]
    so each step reads a contiguous [128, 8] slice.
"""

import numpy as np

SEQ = 512
WDIM, PDIM = 256, 64
H = 256           # per-direction hidden
G = 4 * H         # 1024 gates
MLP = 512
NCORES = 8
HS = SEQ // NCORES  # 64 head rows per core

# psum col j <- weight col-slice start (i0,i1,f0,f1,o0,o1,g0,g1)
JCOL = [0, 128, 256, 384, 768, 896, 512, 640]

_prog_cache = {}


def _build_program(dtw):
    """Build the Bass program. dtw: mybir dtype for recurrent weights/h."""
    from contextlib import ExitStack

    import concourse.bass as bass
    import concourse.mybir as mybir
    import concourse.tile as tile
    from concourse import bacc

    dt = mybir.dt.float32
    T = SEQ
    nc = bacc.Bacc(
        "TRN2", target_bir_lowering=False, debug=False, num_devices=NCORES
    )

    # ---- DRAM I/O ----
    x0t = nc.dram_tensor("x0t", [384, T], dt, kind="ExternalInput")
    wih0t = nc.dram_tensor("wih0t", [2, 384, G], dt, kind="ExternalInput")
    whh0t = nc.dram_tensor("whh0t", [2, 256, G], dt, kind="ExternalInput")
    wih1t = nc.dram_tensor("wih1t", [2, 512, G], dt, kind="ExternalInput")
    whh1t = nc.dram_tensor("whh1t", [2, 256, G], dt, kind="ExternalInput")
    bc0 = nc.dram_tensor("bc0", [2, 128, 8], dt, kind="ExternalInput")
    bc1 = nc.dram_tensor("bc1", [2, 128, 8], dt, kind="ExternalInput")
    w1t = nc.dram_tensor("w1t", [512, MLP], dt, kind="ExternalInput")
    w2t = nc.dram_tensor("w2t", [512, MLP], dt, kind="ExternalInput")
    blin = nc.dram_tensor("blin", [128, 4], dt, kind="ExternalInput")
    wout = nc.dram_tensor("wout", [128, 4], dt, kind="ExternalInput")
    bout = nc.dram_tensor("bout", [128, 1], dt, kind="ExternalInput")
    outd = nc.dram_tensor("out", [HS, T], dt, kind="ExternalOutput")

    Sig = mybir.ActivationFunctionType.Sigmoid
    Tanh = mybir.ActivationFunctionType.Tanh

    with tile.TileContext(nc) as tc, ExitStack() as ctx:
        pers = ctx.enter_context(tc.tile_pool(name="pers", bufs=1))

        # layer outputs, one tile per (dir, khalf): [128, T] each
        l0 = [pers.tile([128, T], dt, tag=f"l0_{i}", name=f"l0_{i}") for i in range(4)]
        l1 = [pers.tile([128, T], dt, tag=f"l1_{i}", name=f"l1_{i}") for i in range(4)]

        def input_gates(xin, KT, wih_dram, bc_dram, tag):
            """Batched x @ Wih.T + b -> gates_sb[d] [128, T*8] t-major."""
            gates_sb = []
            with tc.tile_pool(name=f"ig_{tag}", bufs=1) as igp, \
                 tc.tile_pool(name=f"igp_{tag}", bufs=2,
                              space=bass.MemorySpace.PSUM) as pp:
                for d in range(2):
                    wsb = igp.tile([128, KT * G], dt, tag=f"wsb{d}", name=f"wsb{d}")
                    for k in range(KT):
                        nc.sync.dma_start(
                            wsb[:, G * k:G * (k + 1)],
                            wih_dram[d, 128 * k:128 * (k + 1), :])
                    bsb = igp.tile([128, 8], dt, tag=f"bsb{d}", name=f"bsb{d}")
                    nc.sync.dma_start(bsb[:], bc_dram[d])
                    gsb = pers.tile([128, T * 8], dt, tag=f"gates{d}_{tag}", name=f"gates{d}_{tag}")
                    gv = gsb[:].rearrange("p (t j) -> p t j", j=8)
                    for j in range(8):
                        ps = pp.tile([128, T], dt, name="ig_ps", tag="ig_ps")
                        for k in range(KT):
                            nc.tensor.matmul(
                                ps[:],
                                wsb[:, G * k + JCOL[j]:G * k + JCOL[j] + 128],
                                xin[k][:],
                                start=(k == 0), stop=(k == KT - 1))
                        # evac + bias, strided write into t-major layout
                        nc.vector.tensor_scalar_add(
                            gv[:, :, j], ps[:], bsb[:, j:j + 1])
                    gates_sb.append(gsb)
            return gates_sb

        def lstm_layer(gates_sb, whh_dram, lout, tag):
            with tc.tile_pool(name=f"w_{tag}", bufs=1) as wp, \
                 tc.tile_pool(name=f"s_{tag}", bufs=3) as sp, \
                 tc.tile_pool(name=f"c_{tag}", bufs=4) as cp, \
                 tc.tile_pool(name=f"ps_{tag}", bufs=4,
                              space=bass.MemorySpace.PSUM) as pp:
                whh_sb = []
                for d in range(2):
                    w = wp.tile([128, 2 * G], dtw, tag=f"whh{d}", name=f"whh{d}")
                    if dtw == dt:
                        for k in range(2):
                            nc.sync.dma_start(
                                w[:, G * k:G * (k + 1)],
                                whh_dram[d, 128 * k:128 * (k + 1), :])
                    else:
                        wf = wp.tile([128, 2 * G], dt, tag="whh_f32_stage", name="whh_f32_stage")
                        for k in range(2):
                            nc.sync.dma_start(
                                wf[:, G * k:G * (k + 1)],
                                whh_dram[d, 128 * k:128 * (k + 1), :])
                        nc.vector.tensor_copy(w[:], wf[:])
                    whh_sb.append(w)

                cprev = [None, None]
                for t in range(SEQ):
                    for d in range(2):
                        tcol = t if d == 0 else SEQ - 1 - t
                        gv = gates_sb[d][:, 8 * tcol:8 * tcol + 8]
                        if t == 0:
                            pre = gv
                        else:
                            pcol = tcol - 1 if d == 0 else tcol + 1
                            ps = pp.tile([128, 8], dt, name="lstm_ps", tag="lstm_ps")
                            for j in range(8):
                                for k in range(2):
                                    nc.tensor.matmul(
                                        ps[:, j:j + 1],
                                        whh_sb[d][:, G * k + JCOL[j]:
                                                  G * k + JCOL[j] + 128],
                                        lout[2 * d + k][:, pcol:pcol + 1],
                                        start=(k == 0), stop=(k == 1))
                            tmp = sp.tile([128, 8], dt, tag="tmp", name="tmp")
                            nc.vector.tensor_add(tmp[:], ps[:], gv)
                            pre = tmp[:]
                        sig = sp.tile([128, 8], dt, tag="sig", name="sig")
                        nc.scalar.activation(sig[:, 0:6], pre[:, 0:6], Sig)
                        nc.scalar.activation(sig[:, 6:8], pre[:, 6:8], Tanh)
                        t1 = cp.tile([128, 2], dt, tag="t1", name="t1")
                        nc.vector.tensor_mul(t1[:], sig[:, 0:2], sig[:, 6:8])
                        if t == 0:
                            cnew = t1
                        else:
                            cnew = cp.tile([128, 2], dt, tag="c", name="c")
                            nc.vector.tensor_mul(
                                cnew[:], sig[:, 2:4], cprev[d][:])
                            nc.vector.tensor_add(cnew[:], cnew[:], t1[:])
                        cprev[d] = cnew
                        tct = sp.tile([128, 2], dt, tag="tct", name="tct")
                        nc.scalar.activation(tct[:], cnew[:], Tanh)
                        for k in range(2):
                            nc.vector.tensor_mul(
                                lout[2 * d + k][:, tcol:tcol + 1],
                                sig[:, 4 + k:5 + k], tct[:, k:k + 1])

        # ---- phase A/B: layer 0 ----
        with tc.tile_pool(name="x0", bufs=1) as x0p:
            x0_sb = [x0p.tile([128, SEQ], dt, tag=f"x0_{k}", name=f"x0_{k}") for k in range(3)]
            for k in range(3):
                nc.sync.dma_start(x0_sb[k][:], x0t[128 * k:128 * (k + 1), :])
            g0 = input_gates(x0_sb, 3, wih0t, bc0, "l0")
            lstm_layer(g0, whh0t, l0, "l0")

        # ---- phase C/D: layer 1 ----
        g1 = input_gates(l0, 4, wih1t, bc1, "l1")
        lstm_layer(g1, whh1t, l1, "l1")

        # ---- phase E: u/v projections ----
        with tc.tile_pool(name="uv", bufs=1) as uvp, \
             tc.tile_pool(name="uvps", bufs=2,
                          space=bass.MemorySpace.PSUM) as uvpp:
            w1sb = uvp.tile([128, 4 * MLP], dt, tag="w1sb", name="w1sb")
            w2sb = uvp.tile([128, 4 * MLP], dt, tag="w2sb", name="w2sb")
            for k in range(4):
                nc.sync.dma_start(w1sb[:, MLP * k:MLP * (k + 1)],
                                  w1t[128 * k:128 * (k + 1), :])
                nc.sync.dma_start(w2sb[:, MLP * k:MLP * (k + 1)],
                                  w2t[128 * k:128 * (k + 1), :])
            blin_sb = uvp.tile([128, 4], dt, name="blin_sb", tag="blin_sb")
            nc.sync.dma_start(blin_sb[:], blin[:, :])
            u_sb = pers.tile([128, 4 * SEQ], dt, tag="u_sb", name="u_sb")
            v_sb = pers.tile([128, 4 * SEQ], dt, tag="v_sb", name="v_sb")
            for k in range(4):
                psu = uvpp.tile([128, SEQ], dt, tag="ups", name="ups")
                psv = uvpp.tile([128, SEQ], dt, tag="vps", name="vps")
                for dblk in range(4):
                    nc.tensor.matmul(
                        psu[:],
                        w1sb[:, MLP * dblk + 128 * k:MLP * dblk + 128 * (k + 1)],
                        l1[dblk][:], start=(dblk == 0), stop=(dblk == 3))
                for dblk in range(4):
                    nc.tensor.matmul(
                        psv[:],
                        w2sb[:, MLP * dblk + 128 * k:MLP * dblk + 128 * (k + 1)],
                        l1[dblk][:], start=(dblk == 0), stop=(dblk == 3))
                nc.vector.tensor_scalar_add(
                    u_sb[:, SEQ * k:SEQ * (k + 1)], psu[:],
                    blin_sb[:, k:k + 1])
                nc.vector.tensor_copy(
                    v_sb[:, SEQ * k:SEQ * (k + 1)], psv[:])

        # ---- phase F: per-core slice of u ----
        uloc = pers.tile([128, 4 * HS], dt, tag="uloc", name="uloc")
        pid = nc.sync.partition_id()
        for k in range(4):
            nc.sync.dma_start(
                uloc[:, HS * k:HS * (k + 1)],
                u_sb[:, bass.ds(SEQ * k + pid * HS, HS)])

        # ---- phase G: pair grid ----
        # 4 head rows per PSUM bank at partitions {0,32,64,96} via col-group
        # tile_position; one DVE evac per 4 rows fuses the +b_out.
        with tc.tile_pool(name="grid", bufs=1) as gp, \
             tc.tile_pool(name="pre", bufs=2) as prep, \
             tc.tile_pool(name="th", bufs=5) as thp, \
             tc.tile_pool(name="stg", bufs=2) as stp, \
             tc.tile_pool(name="gps", bufs=4,
                          space=bass.MemorySpace.PSUM) as gpp:
            wout_sb = gp.tile([128, 4], dt, name="wout_sb", tag="wout_sb")
            nc.sync.dma_start(wout_sb[:], wout[:, :])
            bout_sb = gp.tile([128, 1], dt, name="bout_sb", tag="bout_sb")
            nc.sync.dma_start(bout_sb[:], bout[:, :])
            for hg in range(HS // 4):
                ps = gpp.tile([128, SEQ], dt, name="grid_ps", tag="grid_ps")
                for j in range(4):
                    lh = 4 * hg + j
                    pre = prep.tile([128, 4 * SEQ], dt, name="pre", tag="pre")
                    for k in range(4):
                        nc.vector.tensor_scalar_add(
                            pre[:, SEQ * k:SEQ * (k + 1)],
                            v_sb[:, SEQ * k:SEQ * (k + 1)],
                            uloc[:, HS * k + lh:HS * k + lh + 1])
                    th = thp.tile([128, 4 * SEQ], dt, name="th", tag="th")
                    nc.scalar.activation(th[:], pre[:], Tanh)
                    for k in range(4):
                        nc.tensor.matmul(
                            ps[32 * j:32 * j + 1, :], wout_sb[:, k:k + 1],
                            th[:, SEQ * k:SEQ * (k + 1)],
                            start=(k == 0), stop=(k == 3),
                            skip_group_check=True,
                            tile_position=(0, 32 * j))
                stage = stp.tile([128, SEQ], dt, name="stage", tag="stage")
                nc.vector.tensor_scalar_add(stage[:], ps[:], bout_sb[:, 0:1])
                for j in range(4):
                    nc.sync.dma_start(
                        outd[4 * hg + j:4 * hg + j + 1, :],
                        stage[32 * j:32 * j + 1, :])

    nc.compile()
    return nc


def _prep_inputs(inputs):
    f = np.float32
    word_tensor = np.asarray(inputs["word_tensor"])
    pos_tensor = np.asarray(inputs["pos_tensor"])
    word_emb = np.asarray(inputs["word_emb"], f)
    pos_emb = np.asarray(inputs["pos_emb"], f)
    embeds = np.concatenate(
        [word_emb[word_tensor], pos_emb[pos_tensor]], axis=-1)  # [T, 320]

    x0t = np.zeros((384, SEQ), f)
    x0t[:320] = embeds.T

    wih0 = np.asarray(inputs["wih0"], f)
    wih0t = np.zeros((2, 384, G), f)
    for d in range(2):
        wih0t[d, :320] = wih0[d].T
    whh0t = np.ascontiguousarray(
        np.transpose(np.asarray(inputs["whh0"], f), (0, 2, 1)))
    wih1t = np.ascontiguousarray(
        np.transpose(np.asarray(inputs["wih1"], f), (0, 2, 1)))
    whh1t = np.ascontiguousarray(
        np.transpose(np.asarray(inputs["whh1"], f), (0, 2, 1)))

    def bcat(bih, bhh):
        b = np.asarray(bih, f) + np.asarray(bhh, f)  # [2, G]
        out = np.zeros((2, 128, 8), f)
        for d in range(2):
            for j in range(8):
                out[d, :, j] = b[d, JCOL[j]:JCOL[j] + 128]
        return out

    bc0 = bcat(inputs["bih0"], inputs["bhh0"])
    bc1 = bcat(inputs["bih1"], inputs["bhh1"])

    W_lin = np.asarray(inputs["W_lin"], f)  # [MLP, 1024]
    w1t = np.ascontiguousarray(W_lin[:, :512].T)  # [512, MLP]
    w2t = np.ascontiguousarray(W_lin[:, 512:].T)
    b_lin = np.asarray(inputs["b_lin"], f)
    blin = np.zeros((128, 4), f)
    w_out = np.asarray(inputs["w_out"], f)
    wout = np.zeros((128, 4), f)
    for k in range(4):
        blin[:, k] = b_lin[128 * k:128 * (k + 1)]
        wout[:, k] = w_out[0, 128 * k:128 * (k + 1)]
    bout = np.broadcast_to(
        np.asarray(inputs["b_out"], f).reshape(1, 1), (128, 1)).copy()

    return {
        "x0t": x0t, "wih0t": wih0t, "whh0t": whh0t, "wih1t": wih1t,
        "whh1t": whh1t, "bc0": bc0, "bc1": bc1, "w1t": w1t, "w2t": w2t,
        "blin": blin, "wout": wout, "bout": bout,
    }


def kernel(trace=False, **inputs):
    from concourse.bass_utils import run_bass_kernel_spmd

    key = "f32"
    if key not in _prog_cache:
        import concourse.mybir as mybir
        _prog_cache[key] = _build_program(mybir.dt.float32)
    nc = _prog_cache[key]

    in_map = _prep_inputs(inputs)
    res = run_bass_kernel_spmd(
        nc, [dict(in_map) for _ in range(NCORES)],
        core_ids=list(range(NCORES)), trace=trace)

    S = np.concatenate(
        [res.results[i]["out"] for i in range(NCORES)], axis=0)
    S = S.astype(np.float32)
    S[np.eye(SEQ, dtype=bool)] = 0.0
    if trace:
        return S, res
    return S


# revision 17
# speedup vs baseline: 3.9666x; 3.9666x over previous
"""BiLSTM parser kernel for 8 TRN2 NeuronCores.

Strategy (per sharding hint): the sequential 2-layer BiLSTM is replicated on
every core (fwd/bwd interleaved on one core's engines); the n x n pair grid
is sharded by head rows: core i computes score[64*i : 64*i+64, :] using its
partition_id to slice the u-matrix columns. Host gathers the 8 row blocks.

Layouts are feature-on-partition ("transposed") throughout:
  - x.T / lstm_out.T tiles: [128 feat, 512 t]
  - recurrent matmul: weights stationary (orientation B), gates land in one
    PSUM bank as 8 columns [128, 8]; col order (i0,i1,f0,f1,o0,o1,g0,g1) so
    sigmoid covers cols 0:6 in one ACT and tanh cols 6:8.
  - gates_in precomputed batched, stored t-major-interleaved [128, (t,8)]
    so each step reads a contiguous [128, 8] slice.
"""

import numpy as np

SEQ = 512
WDIM, PDIM = 256, 64
H = 256           # per-direction hidden
G = 4 * H         # 1024 gates
MLP = 512
NCORES = 8
HS = SEQ // NCORES  # 64 head rows per core

# psum col j <- weight col-slice start (i0,i1,f0,f1,o0,o1,g0,g1)
JCOL = [0, 128, 256, 384, 768, 896, 512, 640]

_prog_cache = {}


def _build_program(dtw):
    """Build the Bass program. dtw: mybir dtype for recurrent weights/h."""
    from contextlib import ExitStack

    import concourse.bass as bass
    import concourse.mybir as mybir
    import concourse.tile as tile
    from concourse import bacc

    dt = mybir.dt.float32
    T = SEQ
    nc = bacc.Bacc(
        "TRN2", target_bir_lowering=False, debug=False, num_devices=NCORES
    )

    # ---- DRAM I/O ----
    x0t = nc.dram_tensor("x0t", [384, T], dt, kind="ExternalInput")
    wih0t = nc.dram_tensor("wih0t", [2, 384, G], dt, kind="ExternalInput")
    whh0t = nc.dram_tensor("whh0t", [2, 256, G], dt, kind="ExternalInput")
    wih1t = nc.dram_tensor("wih1t", [2, 512, G], dt, kind="ExternalInput")
    whh1t = nc.dram_tensor("whh1t", [2, 256, G], dt, kind="ExternalInput")
    bc0 = nc.dram_tensor("bc0", [2, 128, 8], dt, kind="ExternalInput")
    bc1 = nc.dram_tensor("bc1", [2, 128, 8], dt, kind="ExternalInput")
    w1t = nc.dram_tensor("w1t", [512, MLP], dt, kind="ExternalInput")
    w2t = nc.dram_tensor("w2t", [512, MLP], dt, kind="ExternalInput")
    blin = nc.dram_tensor("blin", [128, 4], dt, kind="ExternalInput")
    wout = nc.dram_tensor("wout", [128, 4], dt, kind="ExternalInput")
    bout = nc.dram_tensor("bout", [128, 1], dt, kind="ExternalInput")
    outd = nc.dram_tensor("out", [HS, T], dt, kind="ExternalOutput")

    Sig = mybir.ActivationFunctionType.Sigmoid
    Tanh = mybir.ActivationFunctionType.Tanh

    with tile.TileContext(nc) as tc, ExitStack() as ctx:
        pers = ctx.enter_context(tc.tile_pool(name="pers", bufs=1))

        # layer outputs, one [128, 2T] tile per dir (k-halves side by side)
        l0m = [pers.tile([128, 2 * T], dt, tag=f"l0m{d}", name=f"l0m{d}")
               for d in range(2)]
        l1m = [pers.tile([128, 2 * T], dt, tag=f"l1m{d}", name=f"l1m{d}")
               for d in range(2)]
        l0 = [l0m[i // 2][:, T * (i % 2):T * (i % 2 + 1)] for i in range(4)]
        l1 = [l1m[i // 2][:, T * (i % 2):T * (i % 2 + 1)] for i in range(4)]

        def input_gates(xin, KT, wih_dram, bc_dram, tag):
            """Batched x @ Wih.T + b -> gates_sb[d] [128, T*8] t-major."""
            gates_sb = []
            with tc.tile_pool(name=f"ig_{tag}", bufs=1) as igp, \
                 tc.tile_pool(name=f"igp_{tag}", bufs=2,
                              space=bass.MemorySpace.PSUM) as pp:
                for d in range(2):
                    wsb = igp.tile([128, KT * G], dt, tag=f"wsb{d}", name=f"wsb{d}")
                    for k in range(KT):
                        nc.sync.dma_start(
                            wsb[:, G * k:G * (k + 1)],
                            wih_dram[d, 128 * k:128 * (k + 1), :])
                    bsb = igp.tile([128, 8], dt, tag=f"bsb{d}", name=f"bsb{d}")
                    nc.sync.dma_start(bsb[:], bc_dram[d])
                    gsb = pers.tile([128, T * 8], dt, tag=f"gates{d}_{tag}", name=f"gates{d}_{tag}")
                    gv = gsb[:].rearrange("p (t j) -> p t j", j=8)
                    for j in range(8):
                        ps = pp.tile([128, T], dt, name="ig_ps", tag="ig_ps")
                        for k in range(KT):
                            nc.tensor.matmul(
                                ps[:],
                                wsb[:, G * k + JCOL[j]:G * k + JCOL[j] + 128],
                                xin[k],
                                start=(k == 0), stop=(k == KT - 1))
                        # evac + bias, strided write into t-major layout
                        nc.vector.tensor_scalar_add(
                            gv[:, :, j], ps[:], bsb[:, j:j + 1])
                    gates_sb.append(gsb)
            return gates_sb

        def lstm_layer(gates_sb, whh_dram, loutm, tag):
            # loutm: per-dir [128, 2T] fp32 output tiles
            with tc.tile_pool(name=f"w_{tag}", bufs=1) as wp, \
                 tc.tile_pool(name=f"s_{tag}", bufs=3) as sp, \
                 tc.tile_pool(name=f"c_{tag}", bufs=4) as cp, \
                 tc.tile_pool(name=f"ps_{tag}", bufs=4,
                              space=bass.MemorySpace.PSUM) as pp:
                whh_sb = []
                hbf = []
                for d in range(2):
                    w = wp.tile([128, 2 * G], dtw, tag=f"whh{d}", name=f"whh{d}")
                    if dtw == dt:
                        for k in range(2):
                            nc.sync.dma_start(
                                w[:, G * k:G * (k + 1)],
                                whh_dram[d, 128 * k:128 * (k + 1), :])
                    else:
                        wf = wp.tile([128, 2 * G], dt, tag="whh_f32_stage", name="whh_f32_stage")
                        for k in range(2):
                            nc.sync.dma_start(
                                wf[:, G * k:G * (k + 1)],
                                whh_dram[d, 128 * k:128 * (k + 1), :])
                        nc.vector.tensor_copy(w[:], wf[:])
                        hb = wp.tile([128, 2 * T], dtw, tag=f"hbf{d}",
                                     name=f"hbf{d}")
                        hbf.append(hb)
                    whh_sb.append(w)

                def hsrc(d, col):
                    if dtw == dt:
                        return loutm[d]
                    return hbf[d]

                cprev = [None, None]
                for t in range(SEQ):
                    for d in range(2):
                        tcol = t if d == 0 else SEQ - 1 - t
                        gv = gates_sb[d][:, 8 * tcol:8 * tcol + 8]
                        if t == 0:
                            pre = gv
                        else:
                            pcol = tcol - 1 if d == 0 else tcol + 1
                            hs = hsrc(d, pcol)
                            ps = pp.tile([128, 8], dt, name="lstm_ps", tag="lstm_ps")
                            for j in range(8):
                                for k in range(2):
                                    nc.tensor.matmul(
                                        ps[:, j:j + 1],
                                        whh_sb[d][:, G * k + JCOL[j]:
                                                  G * k + JCOL[j] + 128],
                                        hs[:, T * k + pcol:T * k + pcol + 1],
                                        start=(k == 0), stop=(k == 1))
                            tmp = sp.tile([128, 8], dt, tag="tmp", name="tmp")
                            nc.vector.tensor_add(tmp[:], ps[:], gv)
                            pre = tmp[:]
                        sig = sp.tile([128, 8], dt, tag="sig", name="sig")
                        # g-gate weights are pre-scaled x2 on host:
                        # tanh(x) = 2*sigmoid(2x) - 1, so one Sigmoid
                        # covers all 8 columns; cols 6:8 fixed up on DVE.
                        nc.scalar.activation(sig[:], pre[:], Sig)
                        nc.vector.tensor_scalar(
                            sig[:, 6:8], sig[:, 6:8], 2.0, -1.0,
                            mybir.AluOpType.mult, mybir.AluOpType.add)
                        t1 = cp.tile([128, 2], dt, tag="t1", name="t1")
                        nc.vector.tensor_mul(t1[:], sig[:, 0:2], sig[:, 6:8])
                        if t == 0:
                            cnew = t1
                        else:
                            cnew = cp.tile([128, 2], dt, tag="c", name="c")
                            nc.vector.tensor_mul(
                                cnew[:], sig[:, 2:4], cprev[d][:])
                            nc.vector.tensor_add(cnew[:], cnew[:], t1[:])
                        cprev[d] = cnew
                        tct = sp.tile([128, 2], dt, tag="tct", name="tct")
                        nc.scalar.activation(tct[:], cnew[:], Tanh)
                        for k in range(2):
                            nc.vector.tensor_mul(
                                loutm[d][:, T * k + tcol:T * k + tcol + 1],
                                sig[:, 4 + k:5 + k], tct[:, k:k + 1])
                        if dtw != dt:
                            nc.vector.tensor_copy(
                                hbf[d][:].rearrange(
                                    "p (k t) -> p t k", k=2)[:, tcol, :],
                                loutm[d][:].rearrange(
                                    "p (k t) -> p t k", k=2)[:, tcol, :])

        # ---- phase A/B: layer 0 ----
        with tc.tile_pool(name="x0", bufs=1) as x0p:
            x0_sb = [x0p.tile([128, SEQ], dt, tag=f"x0_{k}", name=f"x0_{k}") for k in range(3)]
            for k in range(3):
                nc.sync.dma_start(x0_sb[k][:], x0t[128 * k:128 * (k + 1), :])
            g0 = input_gates([t[:] for t in x0_sb], 3, wih0t, bc0, "l0")
            lstm_layer(g0, whh0t, l0m, "l0")

        # ---- phase C/D: layer 1 ----
        g1 = input_gates(l0, 4, wih1t, bc1, "l1")
        lstm_layer(g1, whh1t, l1m, "l1")

        # ---- phase E: u/v projections ----
        with tc.tile_pool(name="uv", bufs=1) as uvp, \
             tc.tile_pool(name="uvps", bufs=2,
                          space=bass.MemorySpace.PSUM) as uvpp:
            w1sb = uvp.tile([128, 4 * MLP], dt, tag="w1sb", name="w1sb")
            w2sb = uvp.tile([128, 4 * MLP], dt, tag="w2sb", name="w2sb")
            for k in range(4):
                nc.sync.dma_start(w1sb[:, MLP * k:MLP * (k + 1)],
                                  w1t[128 * k:128 * (k + 1), :])
                nc.sync.dma_start(w2sb[:, MLP * k:MLP * (k + 1)],
                                  w2t[128 * k:128 * (k + 1), :])
            blin_sb = uvp.tile([128, 4], dt, name="blin_sb", tag="blin_sb")
            nc.sync.dma_start(blin_sb[:], blin[:, :])
            u_sb = pers.tile([128, 4 * SEQ], dt, tag="u_sb", name="u_sb")
            v_sb = pers.tile([128, 4 * SEQ], dt, tag="v_sb", name="v_sb")
            for k in range(4):
                psu = uvpp.tile([128, SEQ], dt, tag="ups", name="ups")
                psv = uvpp.tile([128, SEQ], dt, tag="vps", name="vps")
                for dblk in range(4):
                    nc.tensor.matmul(
                        psu[:],
                        w1sb[:, MLP * dblk + 128 * k:MLP * dblk + 128 * (k + 1)],
                        l1[dblk], start=(dblk == 0), stop=(dblk == 3))
                for dblk in range(4):
                    nc.tensor.matmul(
                        psv[:],
                        w2sb[:, MLP * dblk + 128 * k:MLP * dblk + 128 * (k + 1)],
                        l1[dblk], start=(dblk == 0), stop=(dblk == 3))
                nc.vector.tensor_scalar_add(
                    u_sb[:, SEQ * k:SEQ * (k + 1)], psu[:],
                    blin_sb[:, k:k + 1])
                nc.vector.tensor_copy(
                    v_sb[:, SEQ * k:SEQ * (k + 1)], psv[:])

        # ---- phase F: per-core slice of u ----
        uloc = pers.tile([128, 4 * HS], dt, tag="uloc", name="uloc")
        pid = nc.sync.partition_id()
        for k in range(4):
            nc.sync.dma_start(
                uloc[:, HS * k:HS * (k + 1)],
                u_sb[:, bass.ds(SEQ * k + pid * HS, HS)])

        # ---- phase G: pair grid ----
        # 4 head rows per PSUM bank at partitions {0,32,64,96} via col-group
        # tile_position; one DVE evac per 4 rows fuses the +b_out.
        with tc.tile_pool(name="grid", bufs=1) as gp, \
             tc.tile_pool(name="pre", bufs=2) as prep, \
             tc.tile_pool(name="th", bufs=5) as thp, \
             tc.tile_pool(name="stg", bufs=2) as stp, \
             tc.tile_pool(name="gps", bufs=4,
                          space=bass.MemorySpace.PSUM) as gpp:
            wout_sb = gp.tile([128, 4], dt, name="wout_sb", tag="wout_sb")
            nc.sync.dma_start(wout_sb[:], wout[:, :])
            bout_sb = gp.tile([128, 1], dt, name="bout_sb", tag="bout_sb")
            nc.sync.dma_start(bout_sb[:], bout[:, :])
            for hg in range(HS // 4):
                ps = gpp.tile([128, SEQ], dt, name="grid_ps", tag="grid_ps")
                for j in range(4):
                    lh = 4 * hg + j
                    pre = prep.tile([128, 4 * SEQ], dt, name="pre", tag="pre")
                    for k in range(4):
                        nc.vector.tensor_scalar_add(
                            pre[:, SEQ * k:SEQ * (k + 1)],
                            v_sb[:, SEQ * k:SEQ * (k + 1)],
                            uloc[:, HS * k + lh:HS * k + lh + 1])
                    th = thp.tile([128, 4 * SEQ], dt, name="th", tag="th")
                    nc.scalar.activation(th[:], pre[:], Tanh)
                    for k in range(4):
                        nc.tensor.matmul(
                            ps[32 * j:32 * j + 1, :], wout_sb[:, k:k + 1],
                            th[:, SEQ * k:SEQ * (k + 1)],
                            start=(k == 0), stop=(k == 3),
                            skip_group_check=True,
                            tile_position=(0, 32 * j))
                stage = stp.tile([128, SEQ], dt, name="stage", tag="stage")
                nc.vector.tensor_scalar_add(stage[:], ps[:], bout_sb[:, 0:1])
                for j in range(4):
                    nc.sync.dma_start(
                        outd[4 * hg + j:4 * hg + j + 1, :],
                        stage[32 * j:32 * j + 1, :])

    nc.compile()
    return nc


def _prep_inputs(inputs):
    f = np.float32
    word_tensor = np.asarray(inputs["word_tensor"])
    pos_tensor = np.asarray(inputs["pos_tensor"])
    word_emb = np.asarray(inputs["word_emb"], f)
    pos_emb = np.asarray(inputs["pos_emb"], f)
    embeds = np.concatenate(
        [word_emb[word_tensor], pos_emb[pos_tensor]], axis=-1)  # [T, 320]

    x0t = np.zeros((384, SEQ), f)
    x0t[:320] = embeds.T

    wih0 = np.asarray(inputs["wih0"], f)
    wih0t = np.zeros((2, 384, G), f)
    for d in range(2):
        wih0t[d, :320] = wih0[d].T
    whh0t = np.ascontiguousarray(
        np.transpose(np.asarray(inputs["whh0"], f), (0, 2, 1)))
    wih1t = np.ascontiguousarray(
        np.transpose(np.asarray(inputs["wih1"], f), (0, 2, 1)))
    whh1t = np.ascontiguousarray(
        np.transpose(np.asarray(inputs["whh1"], f), (0, 2, 1)))
    # sigma trick: tanh(x) = 2*sigmoid(2x) - 1 -> g-gate weights x2
    for wt in (wih0t, whh0t, wih1t, whh1t):
        wt[:, :, 512:768] *= 2.0

    def bcat(bih, bhh):
        b = np.asarray(bih, f) + np.asarray(bhh, f)  # [2, G]
        out = np.zeros((2, 128, 8), f)
        for d in range(2):
            for j in range(8):
                out[d, :, j] = b[d, JCOL[j]:JCOL[j] + 128]
        out[:, :, 6:8] *= 2.0  # g-gate bias x2 (sigma trick)
        return out

    bc0 = bcat(inputs["bih0"], inputs["bhh0"])
    bc1 = bcat(inputs["bih1"], inputs["bhh1"])

    W_lin = np.asarray(inputs["W_lin"], f)  # [MLP, 1024]
    w1t = np.ascontiguousarray(W_lin[:, :512].T)  # [512, MLP]
    w2t = np.ascontiguousarray(W_lin[:, 512:].T)
    b_lin = np.asarray(inputs["b_lin"], f)
    blin = np.zeros((128, 4), f)
    w_out = np.asarray(inputs["w_out"], f)
    wout = np.zeros((128, 4), f)
    for k in range(4):
        blin[:, k] = b_lin[128 * k:128 * (k + 1)]
        wout[:, k] = w_out[0, 128 * k:128 * (k + 1)]
    bout = np.broadcast_to(
        np.asarray(inputs["b_out"], f).reshape(1, 1), (128, 1)).copy()

    return {
        "x0t": x0t, "wih0t": wih0t, "whh0t": whh0t, "wih1t": wih1t,
        "whh1t": whh1t, "bc0": bc0, "bc1": bc1, "w1t": w1t, "w2t": w2t,
        "blin": blin, "wout": wout, "bout": bout,
    }


def kernel(trace=False, **inputs):
    from concourse.bass_utils import run_bass_kernel_spmd

    key = "f32"
    if key not in _prog_cache:
        import concourse.mybir as mybir
        _prog_cache[key] = _build_program(mybir.dt.float32)
    nc = _prog_cache[key]

    in_map = _prep_inputs(inputs)
    res = run_bass_kernel_spmd(
        nc, [dict(in_map) for _ in range(NCORES)],
        core_ids=list(range(NCORES)), trace=trace)

    S = np.concatenate(
        [res.results[i]["out"] for i in range(NCORES)], axis=0)
    S = S.astype(np.float32)
    S[np.eye(SEQ, dtype=bool)] = 0.0
    if trace:
        return S, res
    return S


# revision 18
# speedup vs baseline: 18.6240x; 4.6952x over previous
"""BiLSTM parser kernel for 8 TRN2 NeuronCores.

Strategy (per sharding hint): the sequential 2-layer BiLSTM is replicated on
every core (fwd/bwd interleaved on one core's engines); the n x n pair grid
is sharded by head rows: core i computes score[64*i : 64*i+64, :] using its
partition_id to slice the u-matrix columns. Host gathers the 8 row blocks.

Layouts are feature-on-partition ("transposed") throughout:
  - x.T / lstm_out.T tiles: [128 feat, 512 t]
  - recurrent matmul: weights stationary (orientation B), gates land in one
    PSUM bank as 8 columns [128, 8]; col order (i0,i1,f0,f1,o0,o1,g0,g1) so
    sigmoid covers cols 0:6 in one ACT and tanh cols 6:8.
  - gates_in precomputed batched, stored t-major-interleaved [128, (t,8)]
    so each step reads a contiguous [128, 8] slice.
"""

import numpy as np

SEQ = 512
WDIM, PDIM = 256, 64
H = 256           # per-direction hidden
G = 4 * H         # 1024 gates
MLP = 512
NCORES = 8
HS = SEQ // NCORES  # 64 head rows per core

# psum col j <- weight col-slice start (i0,i1,f0,f1,o0,o1,g0,g1)
JCOL = [0, 128, 256, 384, 768, 896, 512, 640]

_prog_cache = {}


def _build_program(dtw):
    """Build the Bass program. dtw: mybir dtype for recurrent weights/h."""
    from contextlib import ExitStack

    import concourse.bass as bass
    import concourse.mybir as mybir
    import concourse.tile as tile
    from concourse import bacc

    dt = mybir.dt.float32
    T = SEQ
    nc = bacc.Bacc(
        "TRN2", target_bir_lowering=False, debug=False, num_devices=NCORES
    )

    # ---- DRAM I/O ----
    x0t = nc.dram_tensor("x0t", [384, T], dt, kind="ExternalInput")
    wih0t = nc.dram_tensor("wih0t", [2, 384, G], dt, kind="ExternalInput")
    whh0t = nc.dram_tensor("whh0t", [2, 256, G], dt, kind="ExternalInput")
    wih1t = nc.dram_tensor("wih1t", [2, 512, G], dt, kind="ExternalInput")
    whh1t = nc.dram_tensor("whh1t", [2, 256, G], dt, kind="ExternalInput")
    bc0 = nc.dram_tensor("bc0", [2, 128, 8], dt, kind="ExternalInput")
    bc1 = nc.dram_tensor("bc1", [2, 128, 8], dt, kind="ExternalInput")
    w1t = nc.dram_tensor("w1t", [512, MLP], dt, kind="ExternalInput")
    w2t = nc.dram_tensor("w2t", [512, MLP], dt, kind="ExternalInput")
    blin = nc.dram_tensor("blin", [128, 4], dt, kind="ExternalInput")
    wout = nc.dram_tensor("wout", [128, 4], dt, kind="ExternalInput")
    bout = nc.dram_tensor("bout", [128, 1], dt, kind="ExternalInput")
    outd = nc.dram_tensor("out", [HS, T], dt, kind="ExternalOutput")

    Sig = mybir.ActivationFunctionType.Sigmoid
    Tanh = mybir.ActivationFunctionType.Tanh

    with tile.TileContext(nc) as tc, ExitStack() as ctx:
        pers = ctx.enter_context(tc.tile_pool(name="pers", bufs=1))

        # layer outputs, one [128, 2T] tile per dir (k-halves side by side)
        l0m = [pers.tile([128, 2 * T], dt, tag=f"l0m{d}", name=f"l0m{d}")
               for d in range(2)]
        l1m = [pers.tile([128, 2 * T], dt, tag=f"l1m{d}", name=f"l1m{d}")
               for d in range(2)]
        l0 = [l0m[i // 2][:, T * (i % 2):T * (i % 2 + 1)] for i in range(4)]
        l1 = [l1m[i // 2][:, T * (i % 2):T * (i % 2 + 1)] for i in range(4)]

        def input_gates(xin, KT, wih_dram, bc_dram, tag):
            """Batched x @ Wih.T + b -> gates_sb[d] [128, T*8] t-major."""
            gates_sb = []
            with tc.tile_pool(name=f"ig_{tag}", bufs=1) as igp, \
                 tc.tile_pool(name=f"igp_{tag}", bufs=2,
                              space=bass.MemorySpace.PSUM) as pp:
                for d in range(2):
                    wsb = igp.tile([128, KT * G], dt, tag=f"wsb{d}", name=f"wsb{d}")
                    for k in range(KT):
                        nc.sync.dma_start(
                            wsb[:, G * k:G * (k + 1)],
                            wih_dram[d, 128 * k:128 * (k + 1), :])
                    bsb = igp.tile([128, 8], dt, tag=f"bsb{d}", name=f"bsb{d}")
                    nc.sync.dma_start(bsb[:], bc_dram[d])
                    gsb = pers.tile([128, T * 8], dt, tag=f"gates{d}_{tag}", name=f"gates{d}_{tag}")
                    gv = gsb[:].rearrange("p (t j) -> p t j", j=8)
                    for j in range(8):
                        ps = pp.tile([128, T], dt, name="ig_ps", tag="ig_ps")
                        for k in range(KT):
                            nc.tensor.matmul(
                                ps[:],
                                wsb[:, G * k + JCOL[j]:G * k + JCOL[j] + 128],
                                xin[k],
                                start=(k == 0), stop=(k == KT - 1))
                        # evac + bias, strided write into t-major layout
                        nc.vector.tensor_scalar_add(
                            gv[:, :, j], ps[:], bsb[:, j:j + 1])
                    gates_sb.append(gsb)
            return gates_sb

        def lstm_layer(gates_sb, whh_dram, loutm, tag):
            # loutm: per-dir [128, 2T] fp32 output tiles
            with tc.tile_pool(name=f"w_{tag}", bufs=1) as wp, \
                 tc.tile_pool(name=f"s_{tag}", bufs=3) as sp, \
                 tc.tile_pool(name=f"c_{tag}", bufs=4) as cp, \
                 tc.tile_pool(name=f"ps_{tag}", bufs=4,
                              space=bass.MemorySpace.PSUM) as pp:
                whh_sb = []
                hbf = []
                for d in range(2):
                    w = wp.tile([128, 2 * G], dtw, tag=f"whh{d}", name=f"whh{d}")
                    if dtw == dt:
                        for k in range(2):
                            nc.sync.dma_start(
                                w[:, G * k:G * (k + 1)],
                                whh_dram[d, 128 * k:128 * (k + 1), :])
                    else:
                        wf = wp.tile([128, 2 * G], dt, tag="whh_f32_stage", name="whh_f32_stage")
                        for k in range(2):
                            nc.sync.dma_start(
                                wf[:, G * k:G * (k + 1)],
                                whh_dram[d, 128 * k:128 * (k + 1), :])
                        nc.vector.tensor_copy(w[:], wf[:])
                        hb = wp.tile([128, 2 * T], dtw, tag=f"hbf{d}",
                                     name=f"hbf{d}")
                        hbf.append(hb)
                    whh_sb.append(w)

                def hsrc(d, col):
                    if dtw == dt:
                        return loutm[d]
                    return hbf[d]

                cprev = [None, None]
                for t in range(SEQ):
                    for d in range(2):
                        tcol = t if d == 0 else SEQ - 1 - t
                        gv = gates_sb[d][:, 8 * tcol:8 * tcol + 8]
                        if t == 0:
                            pre = gv
                        else:
                            pcol = tcol - 1 if d == 0 else tcol + 1
                            hs = hsrc(d, pcol)
                            ps = pp.tile([128, 8], dt, name="lstm_ps", tag="lstm_ps")
                            for j in range(8):
                                for k in range(2):
                                    nc.tensor.matmul(
                                        ps[:, j:j + 1],
                                        whh_sb[d][:, G * k + JCOL[j]:
                                                  G * k + JCOL[j] + 128],
                                        hs[:, T * k + pcol:T * k + pcol + 1],
                                        start=(k == 0), stop=(k == 1))
                            tmp = sp.tile([128, 8], dt, tag="tmp", name="tmp")
                            nc.vector.tensor_add(tmp[:], ps[:], gv)
                            pre = tmp[:]
                        sig = sp.tile([128, 8], dt, tag="sig", name="sig")
                        # g-gate weights are pre-scaled x2 on host:
                        # tanh(x) = 2*sigmoid(2x) - 1, so one Sigmoid
                        # covers all 8 columns; cols 6:8 fixed up on DVE.
                        nc.scalar.activation(sig[:], pre[:], Sig)
                        nc.vector.tensor_scalar(
                            sig[:, 6:8], sig[:, 6:8], 2.0, -1.0,
                            mybir.AluOpType.mult, mybir.AluOpType.add)
                        t1 = cp.tile([128, 2], dt, tag="t1", name="t1")
                        nc.vector.tensor_mul(t1[:], sig[:, 0:2], sig[:, 6:8])
                        if t == 0:
                            cnew = t1
                        else:
                            cnew = cp.tile([128, 2], dt, tag="c", name="c")
                            nc.vector.tensor_mul(
                                cnew[:], sig[:, 2:4], cprev[d][:])
                            nc.vector.tensor_add(cnew[:], cnew[:], t1[:])
                        cprev[d] = cnew
                        tct = sp.tile([128, 2], dt, tag="tct", name="tct")
                        nc.scalar.activation(tct[:], cnew[:], Tanh)
                        for k in range(2):
                            nc.vector.tensor_mul(
                                loutm[d][:, T * k + tcol:T * k + tcol + 1],
                                sig[:, 4 + k:5 + k], tct[:, k:k + 1])
                        if dtw != dt:
                            nc.vector.tensor_copy(
                                hbf[d][:].rearrange(
                                    "p (k t) -> p t k", k=2)[:, tcol, :],
                                loutm[d][:].rearrange(
                                    "p (k t) -> p t k", k=2)[:, tcol, :])

        # ---- phase A/B: layer 0 ----
        with tc.tile_pool(name="x0", bufs=1) as x0p:
            x0_sb = [x0p.tile([128, SEQ], dt, tag=f"x0_{k}", name=f"x0_{k}") for k in range(3)]
            for k in range(3):
                nc.sync.dma_start(x0_sb[k][:], x0t[128 * k:128 * (k + 1), :])
            g0 = input_gates([t[:] for t in x0_sb], 3, wih0t, bc0, "l0")
            lstm_layer(g0, whh0t, l0m, "l0")

        # ---- phase C/D: layer 1 ----
        g1 = input_gates(l0, 4, wih1t, bc1, "l1")
        lstm_layer(g1, whh1t, l1m, "l1")

        # ---- phase E: u/v projections ----
        with tc.tile_pool(name="uv", bufs=1) as uvp, \
             tc.tile_pool(name="uvps", bufs=2,
                          space=bass.MemorySpace.PSUM) as uvpp:
            w1sb = uvp.tile([128, 4 * MLP], dt, tag="w1sb", name="w1sb")
            w2sb = uvp.tile([128, 4 * MLP], dt, tag="w2sb", name="w2sb")
            for k in range(4):
                nc.sync.dma_start(w1sb[:, MLP * k:MLP * (k + 1)],
                                  w1t[128 * k:128 * (k + 1), :])
                nc.sync.dma_start(w2sb[:, MLP * k:MLP * (k + 1)],
                                  w2t[128 * k:128 * (k + 1), :])
            blin_sb = uvp.tile([128, 4], dt, name="blin_sb", tag="blin_sb")
            nc.sync.dma_start(blin_sb[:], blin[:, :])
            u_sb = pers.tile([128, 4 * SEQ], dt, tag="u_sb", name="u_sb")
            v_sb = pers.tile([128, 4 * SEQ], dt, tag="v_sb", name="v_sb")
            for k in range(4):
                psu = uvpp.tile([128, SEQ], dt, tag="ups", name="ups")
                psv = uvpp.tile([128, SEQ], dt, tag="vps", name="vps")
                for dblk in range(4):
                    nc.tensor.matmul(
                        psu[:],
                        w1sb[:, MLP * dblk + 128 * k:MLP * dblk + 128 * (k + 1)],
                        l1[dblk], start=(dblk == 0), stop=(dblk == 3))
                for dblk in range(4):
                    nc.tensor.matmul(
                        psv[:],
                        w2sb[:, MLP * dblk + 128 * k:MLP * dblk + 128 * (k + 1)],
                        l1[dblk], start=(dblk == 0), stop=(dblk == 3))
                nc.vector.tensor_scalar_add(
                    u_sb[:, SEQ * k:SEQ * (k + 1)], psu[:],
                    blin_sb[:, k:k + 1])
                nc.vector.tensor_copy(
                    v_sb[:, SEQ * k:SEQ * (k + 1)], psv[:])

        # ---- phase F: per-core slice of u ----
        uloc = pers.tile([128, 4 * HS], dt, tag="uloc", name="uloc")
        pid = nc.sync.partition_id()
        for k in range(4):
            nc.sync.dma_start(
                uloc[:, HS * k:HS * (k + 1)],
                u_sb[:, bass.ds(SEQ * k + pid * HS, HS)])

        # ---- phase G: pair grid ----
        # 4 head rows per PSUM bank at partitions {0,32,64,96} via col-group
        # tile_position; one DVE evac per 4 rows fuses the +b_out.
        with tc.tile_pool(name="grid", bufs=1) as gp, \
             tc.tile_pool(name="pre", bufs=2) as prep, \
             tc.tile_pool(name="th", bufs=5) as thp, \
             tc.tile_pool(name="stg", bufs=2) as stp, \
             tc.tile_pool(name="gps", bufs=4,
                          space=bass.MemorySpace.PSUM) as gpp:
            wout_sb = gp.tile([128, 4], dt, name="wout_sb", tag="wout_sb")
            nc.sync.dma_start(wout_sb[:], wout[:, :])
            bout_sb = gp.tile([128, 1], dt, name="bout_sb", tag="bout_sb")
            nc.sync.dma_start(bout_sb[:], bout[:, :])
            for hg in range(HS // 4):
                ps = gpp.tile([128, SEQ], dt, name="grid_ps", tag="grid_ps")
                for j in range(4):
                    lh = 4 * hg + j
                    pre = prep.tile([128, 4 * SEQ], dt, name="pre", tag="pre")
                    for k in range(4):
                        nc.vector.tensor_scalar_add(
                            pre[:, SEQ * k:SEQ * (k + 1)],
                            v_sb[:, SEQ * k:SEQ * (k + 1)],
                            uloc[:, HS * k + lh:HS * k + lh + 1])
                    th = thp.tile([128, 4 * SEQ], dt, name="th", tag="th")
                    nc.scalar.activation(th[:], pre[:], Tanh)
                    for k in range(4):
                        nc.tensor.matmul(
                            ps[32 * j:32 * j + 1, :], wout_sb[:, k:k + 1],
                            th[:, SEQ * k:SEQ * (k + 1)],
                            start=(k == 0), stop=(k == 3),
                            skip_group_check=True,
                            tile_position=(0, 32 * j))
                stage = stp.tile([128, SEQ], dt, name="stage", tag="stage")
                nc.vector.tensor_scalar_add(stage[:], ps[:], bout_sb[:, 0:1])
                for j in range(4):
                    nc.sync.dma_start(
                        outd[4 * hg + j:4 * hg + j + 1, :],
                        stage[32 * j:32 * j + 1, :])

    nc.compile()
    return nc


def _prep_inputs(inputs):
    f = np.float32
    word_tensor = np.asarray(inputs["word_tensor"])
    pos_tensor = np.asarray(inputs["pos_tensor"])
    word_emb = np.asarray(inputs["word_emb"], f)
    pos_emb = np.asarray(inputs["pos_emb"], f)
    embeds = np.concatenate(
        [word_emb[word_tensor], pos_emb[pos_tensor]], axis=-1)  # [T, 320]

    x0t = np.zeros((384, SEQ), f)
    x0t[:320] = embeds.T

    wih0 = np.asarray(inputs["wih0"], f)
    wih0t = np.zeros((2, 384, G), f)
    for d in range(2):
        wih0t[d, :320] = wih0[d].T
    whh0t = np.ascontiguousarray(
        np.transpose(np.asarray(inputs["whh0"], f), (0, 2, 1)))
    wih1t = np.ascontiguousarray(
        np.transpose(np.asarray(inputs["wih1"], f), (0, 2, 1)))
    whh1t = np.ascontiguousarray(
        np.transpose(np.asarray(inputs["whh1"], f), (0, 2, 1)))
    # sigma trick: tanh(x) = 2*sigmoid(2x) - 1 -> g-gate weights x2
    for wt in (wih0t, whh0t, wih1t, whh1t):
        wt[:, :, 512:768] *= 2.0

    def bcat(bih, bhh):
        b = np.asarray(bih, f) + np.asarray(bhh, f)  # [2, G]
        out = np.zeros((2, 128, 8), f)
        for d in range(2):
            for j in range(8):
                out[d, :, j] = b[d, JCOL[j]:JCOL[j] + 128]
        out[:, :, 6:8] *= 2.0  # g-gate bias x2 (sigma trick)
        return out

    bc0 = bcat(inputs["bih0"], inputs["bhh0"])
    bc1 = bcat(inputs["bih1"], inputs["bhh1"])

    W_lin = np.asarray(inputs["W_lin"], f)  # [MLP, 1024]
    w1t = np.ascontiguousarray(W_lin[:, :512].T)  # [512, MLP]
    w2t = np.ascontiguousarray(W_lin[:, 512:].T)
    b_lin = np.asarray(inputs["b_lin"], f)
    blin = np.zeros((128, 4), f)
    w_out = np.asarray(inputs["w_out"], f)
    wout = np.zeros((128, 4), f)
    for k in range(4):
        blin[:, k] = b_lin[128 * k:128 * (k + 1)]
        wout[:, k] = w_out[0, 128 * k:128 * (k + 1)]
    bout = np.broadcast_to(
        np.asarray(inputs["b_out"], f).reshape(1, 1), (128, 1)).copy()

    return {
        "x0t": x0t, "wih0t": wih0t, "whh0t": whh0t, "wih1t": wih1t,
        "whh1t": whh1t, "bc0": bc0, "bc1": bc1, "w1t": w1t, "w2t": w2t,
        "blin": blin, "wout": wout, "bout": bout,
    }


def kernel(trace=False, **inputs):
    from concourse.bass_utils import run_bass_kernel_spmd

    import os
    import concourse.mybir as mybir
    key = os.environ.get("KERNEL_RECUR_DTYPE", "bf16")
    if key not in _prog_cache:
        dtw = mybir.dt.float32 if key == "f32" else mybir.dt.bfloat16
        _prog_cache[key] = _build_program(dtw)
    nc = _prog_cache[key]

    in_map = _prep_inputs(inputs)
    res = run_bass_kernel_spmd(
        nc, [dict(in_map) for _ in range(NCORES)],
        core_ids=list(range(NCORES)), trace=trace)

    S = np.concatenate(
        [res.results[i]["out"] for i in range(NCORES)], axis=0)
    S = S.astype(np.float32)
    S[np.eye(SEQ, dtype=bool)] = 0.0
    if trace:
        return S, res
    return S
